# revision 1
# baseline (speedup 1.0000x reference)
"""CRF forward-algorithm (log partition) kernel for Trainium2, 8 NeuronCores.

Math
----
Reference computes, per batch element b with feats f[b,s,i], transitions
A[i,j], contiguous-prefix mask of length L[b]:

    score_0 = onehot(START) in log space
    score_{s+1}[i] = LSE_j(score_s[j] + A[i,j]) + f[b,s,i]      (while s < L)
    logZ[b] = LSE_i(score_{L}[i] + A[STOP, i])

We run the recurrence in *linear* space:  P_s = exp(score_s) (periodically
rescaled).  One step is  P_{s+1} = (E @ P_s) * ef_s  with E = exp(A) and
ef_s = exp(f[:, s, :]).  Per core (128 batch elements on the free dim,
T=48 tags on the partition dim) each step is ONE TensorE matmul with a
static [48, 96] stationary matrix:

    cols 0..47  : E^T        -> rows 0..47  = E @ P          (the update)
    cols 48..95 : exp(A[STOP,:]) replicated
                 -> rows 48..95 = w = sum_j P[j] * exp(A[STOP,j])
                    (row 48: per-step readout; rows 48..95: broadcast
                     normalizer for the periodic rescale)

followed by ONE VectorE multiply with ef_s.  Every RESC steps P is also
multiplied by 1/w (rows 48..95 give the [48,128] broadcast for free).

Masking never touches the device: masks are contiguous prefixes, so
logZ[b] is just the readout after L[b] steps.  The device stores the raw
readout w_s for every s (plus implicitly the rescale history, which is
the same W rows at the rescale steps), and the host reconstructs

    logZ[b] = log W[L[b], b] + sum_{rescale steps s' < L[b]} log W[s', b]

in float64.
"""

import os
import sys

import numpy as np

for _p in ("/opt/trn_rl_repo",):
    if _p not in sys.path and os.path.isdir(_p):
        sys.path.insert(0, _p)

import ml_dtypes  # noqa: E402

import concourse.bass as bass  # noqa: E402
import concourse.bacc as bacc  # noqa: E402
import concourse.mybir as mybir  # noqa: E402
from concourse import tile  # noqa: E402
from concourse.bass_utils import run_bass_kernel_spmd  # noqa: E402

BF16 = ml_dtypes.bfloat16

B, S, T = 1024, 512, 48
NCORES = 8
BSH = B // NCORES  # 128 batch elements per core
START_IDX, STOP_IDX = 45, 46
RESC = 8  # rescale every RESC steps
RECIP_FAST = os.environ.get("CRF_RECIP_FAST", "1") == "1"
CH = 64  # W-readout rows buffered in SBUF before DMA out
NMM = S + 1  # 513 readouts (after 0..512 steps)
RD = 64  # psum partition row holding the w readout (32-aligned for engines)
MAGIC = 0x7EF00000  # fp32 magic reciprocal: bits(r) = MAGIC - bits(w)


def magic_recip(w):
    """Exact replica of the device's one-op approximate reciprocal:
    r = bitcast(MAGIC - bits(w)) ~= 1/w (+-15%) for w in the normal range."""
    bits = np.ascontiguousarray(w, dtype=np.float32).view(np.uint32)
    return (np.uint32(MAGIC) - bits).view(np.float32)


def saved_steps(n_steps=S, resc=RESC):
    """All readout steps are stored (the PSUM-window flush copies whole
    windows, so sparse saving would not be cheaper)."""
    return list(range(n_steps + 1))


def build_nc(
    dtype=mybir.dt.bfloat16, n_steps=S, resc=RESC, ch=CH, chains=2, recip="magic"
):
    f32 = mybir.dt.float32
    nmm = n_steps + 1
    assert n_steps % ch == 0
    assert BSH % chains == 0
    gb = BSH // chains
    # Steps per PSUM window: consecutive matmuls write adjacent column
    # blocks of one PSUM bank, so the (expensive, fixed-cost) ScalarE
    # readout copy runs once per window instead of once per step.
    win = max(1, min(2048 // (gb * 4), ch))
    assert ch % win == 0
    nc = bacc.Bacc("TRN2", target_bir_lowering=False, debug=False)
    eft_d = nc.declare_dram_parameter("eft", [T, n_steps * BSH], dtype, isOutput=False)
    wmat_d = nc.declare_dram_parameter("wmat", [T, 128], dtype, isOutput=False)
    p0_d = nc.declare_dram_parameter("p0", [T, BSH], dtype, isOutput=False)
    cst_d = nc.declare_dram_parameter("cst", [T, BSH], f32, isOutput=False)
    w_d = nc.declare_dram_parameter("W", [nmm, BSH], f32, isOutput=True)

    with tile.TileContext(nc) as tc:
        with (
            tc.tile_pool(name="const", bufs=1) as constp,
            tc.tile_pool(name="eft", bufs=1) as eftp,
            tc.tile_pool(name="state", bufs=4) as statep,
            tc.tile_pool(name="wb", bufs=2) as wbp,
            tc.tile_pool(name="ps", bufs=2, space="PSUM") as psp,
        ):
            wmat_t = constp.tile([T, 128], dtype, tag="wmat")
            nc.sync.dma_start(wmat_t[:], wmat_d[:])
            cst_t = constp.tile([T, BSH], f32, tag="cst")
            nc.sync.dma_start(cst_t[:], cst_d[:])

            eft_tiles = []
            for ci in range(n_steps // ch):
                t = eftp.tile([T, ch * BSH], dtype, tag=f"eft{ci}")
                nc.sync.dma_start(
                    t[:], eft_d[:, ci * ch * BSH : (ci + 1) * ch * BSH]
                )
                eft_tiles.append(t)

            pinit = statep.tile([T, BSH], dtype, tag="pinit")
            nc.sync.dma_start(pinit[:], p0_d[:])
            p_cur = [pinit[:, g * gb : (g + 1) * gb] for g in range(chains)]

            wchunk = None
            ps_win = [None] * chains
            r_pend = [None] * chains
            for s in range(nmm):
                wi, wo = divmod(s, win)
                is_resc = s < n_steps and s % resc == resc - 1
                pre_resc = s + 1 < n_steps and (s + 1) % resc == resc - 1
                for g in range(chains):
                    if wo == 0:
                        ps_t = psp.tile([128, win * gb], f32, tag=f"ps{g}")
                        ps_win[g] = ps_t
                    nc.tensor.matmul(
                        ps_win[g][:, wo * gb : (wo + 1) * gb],
                        wmat_t[:],
                        p_cur[g],
                        start=True,
                        stop=True,
                    )

                # Recurrence first: the W-readout copy shares the PSUM bank
                # and must not serialize ahead of the critical-path multiply.
                if s < n_steps:
                    for g in range(chains):
                        u_ap = ps_win[g][0:T, wo * gb : (wo + 1) * gb]
                        wb_ap = ps_win[g][RD : RD + T, wo * gb : (wo + 1) * gb]
                        c0 = (s % ch) * BSH + g * gb
                        eft_ap = eft_tiles[s // ch][:, c0 : c0 + gb]
                        p_next = statep.tile([T, gb], dtype, tag=f"p{g}")
                        if is_resc:
                            # Rescale using r ~= 1/w computed one step ago
                            # (off the critical path): pre-scale ef by r,
                            # then the usual single multiply.
                            efr = statep.tile([T, gb], f32, tag=f"efr{g}")
                            nc.vector.tensor_mul(efr[:], eft_ap, r_pend[g][:])
                            nc.vector.tensor_mul(p_next[:], u_ap, efr[:])
                        else:
                            nc.vector.tensor_mul(p_next[:], u_ap, eft_ap)
                        if pre_resc:
                            # r ~= 1/w via the fp32 magic-number trick in one
                            # integer op: bits(r) = MAGIC - bits(w).  Only
                            # ~15% accurate, but the host replays the exact
                            # bits, so the bookkeeping is exact.
                            r_t = statep.tile([T, gb], f32, tag=f"r{g}")
                            if recip == "magic":
                                nc.vector.tensor_tensor(
                                    r_t[:].bitcast(mybir.dt.uint32),
                                    cst_t[:, 0:gb].bitcast(mybir.dt.uint32),
                                    wb_ap.bitcast(mybir.dt.uint32),
                                    op=mybir.AluOpType.subtract,
                                )
                            else:
                                nc.vector.reciprocal(r_t[:], wb_ap)
                            r_pend[g] = r_t
                        p_cur[g] = p_next[:]

                # Window flush: one ScalarE copy per chain moves the whole
                # window of w-readout rows PSUM -> SBUF; DMA out per chunk.
                if (s % ch) == 0:
                    wchunk = wbp.tile([RD + 1, ch, BSH], f32, tag="wb")
                if wo == win - 1 or s == nmm - 1:
                    nw = wo + 1
                    co0 = (wi * win) % ch
                    for g in range(chains):
                        nc.scalar.activation(
                            wchunk[RD : RD + 1, co0 : co0 + nw, g * gb : (g + 1) * gb],
                            ps_win[g][RD : RD + 1, 0 : nw * gb].rearrange(
                                "p (k j) -> p k j", j=gb
                            ),
                            mybir.ActivationFunctionType.Copy,
                        )
                if (s % ch) == ch - 1 or s == nmm - 1:
                    ci = s // ch
                    nrows = (s % ch) + 1
                    nc.sync.dma_start(
                        w_d[ci * ch : ci * ch + nrows, :],
                        wchunk[RD : RD + 1, 0:nrows, :],
                    )
    nc.compile()
    return nc


def host_prep(feats, transitions):
    """Returns (per-core eft arrays [T, S*BSH] bf16, wmat [T, 2T] bf16)."""
    E = np.exp(transitions.astype(np.float64))  # [T,T], exp(-10000) -> 0
    wmat = np.zeros((T, 128), np.float64)
    wmat[:, :T] = E.T  # wmat[j, i] = E[i, j]  -> psum rows 0..47 = E @ P
    wmat[:, RD : RD + T] = E[STOP_IDX, :][:, None]  # rows 64..111 = w bcast
    wmat_bf = wmat.astype(BF16)

    ef = np.exp(feats.astype(np.float32))  # [B, S, T]
    efts = []
    for c in range(NCORES):
        sl = ef[c * BSH : (c + 1) * BSH]  # [BSH, S, T]
        eft = np.ascontiguousarray(sl.transpose(2, 1, 0))  # [T, S, BSH]
        efts.append(eft.reshape(T, S * BSH).astype(BF16))
    return efts, wmat_bf


def host_finish(W_all, lengths, n_steps=S, resc=RESC, recip="magic"):
    """W_all: [NCORES, nsaved, BSH] f32 device readouts (saved-step order)."""
    saved = saved_steps(n_steps, resc)
    pos_of = {s: k for k, s in enumerate(saved)}
    logW = np.log(W_all.astype(np.float64))  # [NCORES, nsaved, BSH]
    # The rescale at step s uses the readout of step s-1 (stale-by-one so
    # the reciprocal runs off the critical path).
    resc_pos = np.array(
        [pos_of[s - 1] for s in range(resc - 1, n_steps, resc)]
    )  # positions of the rows whose w fed each rescale factor
    ncores = W_all.shape[0]
    bsh = W_all.shape[2]
    # The device multiplied P by (approximately) 1/w at each rescale step;
    # undo with the exactly-replayed factor.
    if recip == "magic":
        resc_log = -np.log(
            magic_recip(W_all[:, resc_pos, :]).astype(np.float64)
        )  # [ncores, n_resc, bsh]
    else:
        resc_log = logW[:, resc_pos, :]
    cum = np.concatenate(
        [np.zeros((ncores, 1, bsh)), np.cumsum(resc_log, axis=1)], axis=1
    )  # [ncores, n_resc + 1, bsh]
    nb = ncores * bsh
    out = np.empty((nb,), np.float32)
    idx = np.arange(bsh)
    for c in range(ncores):
        Lc = lengths[c * bsh : (c + 1) * bsh]
        Lpos = np.array([pos_of[L] for L in Lc])
        out[c * bsh : (c + 1) * bsh] = (
            logW[c, Lpos, idx] + cum[c, Lc // resc, idx]
        ).astype(np.float32)
    return out


def _run(feats, transitions, masks, trace=False):
    feats = np.asarray(feats)
    transitions = np.asarray(transitions)
    masks = np.asarray(masks)
    lengths = masks.sum(axis=1).astype(np.int64)  # [B], in [S//2, S]

    efts, wmat_bf = host_prep(feats, transitions)
    p0 = np.zeros((T, BSH), np.float32)
    p0[START_IDX, :] = 1.0
    p0 = p0.astype(BF16)
    cst = np.full((T, BSH), MAGIC, np.uint32).view(np.float32)
    in_maps = [
        {"eft": efts[c], "wmat": wmat_bf, "p0": p0, "cst": cst}
        for c in range(NCORES)
    ]

    nc = build_nc(
        chains=int(os.environ.get("CRF_CHAINS", "2")),
        recip=os.environ.get("CRF_RECIP", "magic"),
    )
    bres = run_bass_kernel_spmd(
        nc, in_maps, core_ids=list(range(NCORES)), trace=trace
    )
    W_all = np.stack([r["W"] for r in bres.results])  # [NCORES, NMM, BSH]
    return host_finish(W_all, lengths, recip=os.environ.get("CRF_RECIP", "magic")), bres


def kernel(feats, transitions, masks):
    out, _ = _run(feats, transitions, masks, trace=False)
    return out

